# revision 7
# baseline (speedup 1.0000x reference)
"""CRF loss (mean(log_Z - gold_score)) on 8 Trainium2 NeuronCores.

Strategy:
  - Data-parallel: batch 256 -> 32 per core.
  - log-partition via forward algorithm in exp domain:
        A_t = EE_t * (ET^T A_{t-1}),  EE_t = exp(em_t - shift), ET = exp(trans)
    computed as PE matmul (block-diag stationary ET for 2 partition groups of
    64 tags) + DVE elementwise multiply.
  - The sequential 1023-step scan is broken into C parallel chunks per core.
    Transition mixing (Birkhoff contraction ~ tanh(range(trans)/2) ~ 0.35 per
    step) makes the forward direction forget its initial condition; each chunk
    warms up for W throwaway steps from a uniform vector, after which its
    direction equals the true forward vector to ~0.35^W relative error.
    Chunk log-gains are captured via colsum matmuls + Ln and telescoped on the
    host into log_Z exactly (scale-invariant per chunk).
  - gold score (O(B*S) gathers) + final mean on host.
"""

import numpy as np
import ml_dtypes

NCORES = 8
B, S, T = 256, 1024, 64
BL = B // NCORES          # batch per core
SHIFT = 4.66              # ~E[log growth per step]; keeps exp-domain values ~1

# tunable device config
CFG = dict(C=16, W=32, dt="float32", bs=12)

_cache = {}


def _build_nc(C, W, dt_name, bs, S_=S, BL_=BL):
    """Build the per-core Bass program. Returns (nc, meta)."""
    import concourse.bacc as bacc
    import concourse.tile as tile
    import concourse.mybir as mybir

    G = 2
    CG = C // G
    w = CG * BL_                   # columns of the scan tile
    L = S_ // C                    # owned steps per chunk
    D = W + L                      # super-steps
    f32 = mybir.dt.float32
    dt = {"float32": mybir.dt.float32, "bfloat16": mybir.dt.bfloat16}[dt_name]
    nblk = (D + bs - 1) // bs

    nc = bacc.Bacc("TRN2", target_bir_lowering=False, debug=False,
                   num_devices=NCORES)

    em_raw = nc.declare_dram_parameter("em_raw", [128, D * w], dt, isOutput=False)
    trans_blk = nc.declare_dram_parameter("trans_blk", [128, 128], dt, isOutput=False)
    cap_w = nc.declare_dram_parameter("cap_w", [128, 4], dt, isOutput=False)
    inj = nc.declare_dram_parameter("inj", [64, BL_], dt, isOutput=False)
    out = nc.declare_dram_parameter("out", [12, w], f32, isOutput=True)

    with tile.TileContext(nc) as tc:
        with (
            tc.tile_pool(name="const", bufs=1) as constp,
            tc.tile_pool(name="stage", bufs=2) as stagep,
            tc.tile_pool(name="ee", bufs=nblk) as eep,
            tc.tile_pool(name="a", bufs=3) as ap_,
            tc.tile_pool(name="outp", bufs=1) as outp,
            tc.tile_pool(name="ps", bufs=4, space="PSUM") as psp,
            tc.tile_pool(name="pscap", bufs=3, space="PSUM") as pscapp,
        ):
            trans_t = constp.tile([128, 128], dt, tag="trans")
            nc.sync.dma_start(trans_t[:], trans_blk[:])
            cap_t = constp.tile([128, 4], dt, tag="cap")
            nc.sync.dma_start(cap_t[:], cap_w[:])
            inj_t = constp.tile([64, BL_], dt, tag="inj")
            nc.sync.dma_start(inj_t[:], inj[:])
            out_ts = {r: outp.tile([4, w], f32, name=f"out{r}", tag=f"out{r}")
                      for r in (0, 4, 8)}
            bias_t = constp.tile([128, 1], f32, tag="bias")
            nc.vector.memset(bias_t[:], -SHIFT)

            # EE pipeline: DMA raw -> exp -> resident block tiles
            ee_blocks = []
            for k in range(nblk):
                lo = k * bs
                hi = min(D, lo + bs)
                ncols = (hi - lo) * w
                st = stagep.tile([128, bs * w], dt, tag="stage")
                nc.sync.dma_start(st[:, :ncols], em_raw[:, lo * w:hi * w])
                ee = eep.tile([128, bs * w], dt, tag="ee")
                nc.scalar.activation(ee[:, :ncols], st[:, :ncols],
                                     mybir.ActivationFunctionType.Exp,
                                     bias=bias_t[:])
                ee_blocks.append(ee)

            # initial state: ones
            a_prev = ap_.tile([128, w], dt, tag="a")
            nc.vector.memset(a_prev[:], 1.0)

            def capture(a_cur, row):
                cp = pscapp.tile([4, w], f32, tag="cap")
                nc.tensor.matmul(cp[:], cap_t[:], a_cur[:], start=True, stop=True)
                nc.scalar.activation(out_ts[row][:], cp[:],
                                     mybir.ActivationFunctionType.Ln)

            for u in range(D):
                p = psp.tile([128, w], f32, tag="p")
                nc.tensor.matmul(p[:], trans_t[:], a_prev[:], start=True, stop=True)
                a_new = ap_.tile([128, w], dt, tag="a")
                blk, off = divmod(u, bs)
                nc.vector.tensor_mul(a_new[:], p[:],
                                     ee_blocks[blk][:, off * w:(off + 1) * w])
                if u == W - 1:
                    # overwrite chunk-0 columns with true alpha_0
                    nc.scalar.copy(a_new[0:64, 0:BL_], inj_t[:])
                    capture(a_new, 0)     # baseline
                if u == D - 2:
                    capture(a_new, 4)     # early end (for chunk 0)
                if u == D - 1:
                    capture(a_new, 8)     # late end (+ end-weighted rows)
                a_prev = a_new

            for r, t in out_ts.items():
                nc.sync.dma_start(out[r:r + 4, :], t[:])

    nc.compile()
    meta = dict(C=C, W=W, G=G, CG=CG, w=w, L=L, D=D, dt_name=dt_name)
    return nc, meta


def _np_dt(dt_name):
    return {"float32": np.float32, "bfloat16": ml_dtypes.bfloat16}[dt_name]


def _t_index(C, W, L, D):
    """T_idx[c, u] = emission step index for chunk c at super-step u."""
    T_idx = np.zeros((C, D), dtype=np.int64)
    for c in range(C):
        for u in range(D):
            if c == 0:
                t = u - W + 1
            else:
                t = c * L - W + u
            T_idx[c, u] = t
    return np.clip(T_idx, 1, S - 1)  # bogus slots -> any valid finite step


def _host_inputs(em_l, transitions, start_transitions, end_transitions, meta):
    """Build the per-core DRAM inputs from this core's emissions shard."""
    C, W, G, CG, w, L, D = (meta[k] for k in ("C", "W", "G", "CG", "w", "L", "D"))
    dtn = _np_dt(meta["dt_name"])
    BL_ = em_l.shape[0]
    T_idx = _t_index(C, W, L, D)

    g = em_l[:, T_idx, :]                       # [BL, C, D, T]
    g = g.reshape(BL_, G, CG, D, T)
    g = g.transpose(1, 4, 3, 2, 0)              # [G, T, D, CG, BL]
    em_raw = np.ascontiguousarray(g.reshape(128, D * w)).astype(dtn)

    ET = np.exp(transitions).astype(np.float64)
    trans_blk = np.zeros((128, 128), np.float64)
    trans_blk[0:64, 0:64] = ET
    trans_blk[64:128, 64:128] = ET
    trans_blk = trans_blk.astype(dtn)

    cap_w = np.zeros((128, 4), np.float64)
    cap_w[0:64, 0] = 1.0
    cap_w[64:128, 1] = 1.0
    cap_w[0:64, 2] = np.exp(end_transitions)
    cap_w[64:128, 3] = np.exp(end_transitions)
    cap_w = cap_w.astype(dtn)

    inj = np.exp(start_transitions[:, None] + em_l[:, 0, :].T - SHIFT).astype(dtn)

    return dict(em_raw=em_raw, trans_blk=trans_blk, cap_w=cap_w, inj=inj)


def _assemble_logZ(out, meta):
    """out: [12, w] f32 device output for one core -> logZ [BL] float64."""
    C, W, G, CG, w, L = (meta[k] for k in ("C", "W", "G", "CG", "w", "L"))
    out = out.astype(np.float64)
    logZ = np.zeros(BL)
    for b in range(BL):
        total = 0.0
        for c in range(C):
            g, k = divmod(c, CG)
            x = k * BL + b
            base = out[0 + g, x]
            if c == 0:
                total += out[4 + g, x] - base + (L - 1) * SHIFT
                total += base + SHIFT          # log||alpha_0||
            else:
                total += out[8 + g, x] - base + L * SHIFT
            if c == C - 1:
                total += out[10 + g, x] - out[8 + g, x]
        logZ[b] = total
    return logZ


def _gold_score(emissions, tags, maskf, transitions, start_transitions,
                end_transitions):
    em = emissions.astype(np.float64)
    tr = transitions.astype(np.float64)
    tg = tags.astype(np.int64)
    emit = np.take_along_axis(em, tg[:, :, None], axis=2)[:, :, 0]
    trans = tr[tg[:, :-1], tg[:, 1:]]
    score = start_transitions.astype(np.float64)[tg[:, 0]] + emit[:, 0]
    score = score + np.sum((trans + emit[:, 1:]) * maskf[:, 1:], axis=1)
    last_pos = maskf.astype(np.int64).sum(axis=1) - 1
    last_tags = np.take_along_axis(tg, last_pos[:, None], axis=1)[:, 0]
    return score + end_transitions.astype(np.float64)[last_tags]


def _ref_numpy(emissions, tags, mask, transitions, start_transitions,
               end_transitions):
    """Full-precision host fallback (general mask)."""
    em = emissions.astype(np.float64)
    maskf = mask.astype(np.float64)
    tr = transitions.astype(np.float64)
    alpha = start_transitions.astype(np.float64)[None, :] + em[:, 0]
    for t in range(1, em.shape[1]):
        sc = alpha[:, :, None] + tr[None, :, :] + em[:, t][:, None, :]
        m = sc.max(axis=1)
        new = m + np.log(np.exp(sc - m[:, None, :]).sum(axis=1))
        alpha = np.where(maskf[:, t][:, None] > 0, new, alpha)
    x = alpha + end_transitions.astype(np.float64)[None, :]
    m = x.max(axis=1)
    logZ = m + np.log(np.exp(x - m[:, None]).sum(axis=1))
    score = _gold_score(em, tags, maskf, tr, start_transitions, end_transitions)
    return np.float32(np.mean(logZ - score))


def _get_nc():
    key = (CFG["C"], CFG["W"], CFG["dt"], CFG["bs"])
    if key not in _cache:
        _cache[key] = _build_nc(*key)
    return _cache[key]


def run_device_logZ(emissions):
    """Run the Bass kernel on 8 cores; return logZ [B] float64."""
    from concourse.bass_utils import run_bass_kernel_spmd
    nc, meta = _get_nc()
    em = np.asarray(emissions, dtype=np.float32)
    in_maps = []
    for k in range(NCORES):
        em_l = em[k * BL:(k + 1) * BL]
        in_maps.append(_host_inputs(em_l, run_device_logZ._tr,
                                    run_device_logZ._st, run_device_logZ._en,
                                    meta))
    res = run_bass_kernel_spmd(nc, in_maps, list(range(NCORES)))
    logZ = np.concatenate([_assemble_logZ(res.results[k]["out"], meta)
                           for k in range(NCORES)])
    return logZ


def kernel(emissions, tags, mask, transitions, start_transitions,
           end_transitions):
    emissions = np.asarray(emissions)
    tags = np.asarray(tags)
    mask = np.asarray(mask)
    transitions = np.asarray(transitions)
    start_transitions = np.asarray(start_transitions)
    end_transitions = np.asarray(end_transitions)

    if not np.all(mask == 1):
        return _ref_numpy(emissions, tags, mask, transitions,
                          start_transitions, end_transitions)

    run_device_logZ._tr = transitions.astype(np.float64)
    run_device_logZ._st = start_transitions.astype(np.float64)
    run_device_logZ._en = end_transitions.astype(np.float64)
    logZ = run_device_logZ(emissions)

    maskf = mask.astype(np.float64)
    score = _gold_score(emissions, tags, maskf, transitions,
                        start_transitions, end_transitions)
    return np.float32(np.mean(logZ - score))
